# revision 12
# baseline (speedup 1.0000x reference)
"""Trainium2 Bass kernel for BasicMGU (nn_BasicMGU_53386443489965).

Math (per reference):
    xz = x @ W_k ; xh = x @ W_u
    f_t = sigmoid(xz_t + h @ W_r + b_r)
    c_t = tanh(xh_t + (h*f_t) @ W_ur + b_ur)
    h   = (1-f_t)*h + f_t*c_t        -> return final h  [B, U]

Sharding: data-parallel over batch across 8 cores (B=64 -> 8 per core),
weights replicated.

Per-core design (v5):
  The input projections (two bf16 GEMMs producing bf16 xz/xh slabs) are
  FOLDED INTO the recurrence loop: while the recurrence consumes chunk c,
  the PE's idle windows project chunk c+2 (one N=512 matmul or drain per
  step, emitted after the step's bursts so it lands in the sigmoid-wait
  window).  Only chunks 0/1 are projected up front (~25us).
  The hw loop body covers a chunk PAIR (A/B), with two statically-placed
  slab tiles: A's slab for the next pair is DMA-loaded right after A's
  last read (mid-body), so chunk heads never wait on DMA, and the
  boundary-reset machinery runs half as often.
  Phase 2 (recurrence): state kept transposed hT [U(part), B(free)].
  Both per-step matmuls run weight-stationary (lhsT = 128x128 weight
  tile in bf16 -> fast weight load, rhs = state in bf16, N=B=8).
  Accumulation groups stay consecutive per PSUM slice; different PSUM
  tiles may interleave.  All compute-engine APs are static (register
  offsets cost ~100-270ns per use on the busy queues); register offsets
  appear only in DMA instructions on the sync queue.
"""

import os
import sys
import types

sys.path.insert(0, "/opt/trn_rl_repo")

import numpy as np
import ml_dtypes

import concourse.bass as bass
import concourse.mybir as mybir
import concourse.tile as tile
from concourse import bacc
from concourse.bass_utils import run_bass_kernel_spmd

B, T, D, U = 64, 1024, 512, 512
NCORES = 8
BL = B // NCORES          # batch per core
S = 64                    # recurrence steps per chunk
NCH = T // S              # chunks
KC = D // 128             # contraction chunks
MC = U // 128             # output-unit chunks
PCOLS = S * BL            # projection (t,b) columns per chunk block
NW = S * BL               # free width of one swizzled chunk slab
PAD_BLKS = 2              # scratch projection targets for the last body

F32 = mybir.dt.float32
BF16 = mybir.dt.bfloat16

LAST_EXEC_NS = None


def _install_trace_shim():
    """Make `antenv.axon_hooks` importable so trace=True degrades gracefully
    (and, where the axon .so is present, actually captures NTFF profiles)."""
    if "antenv.axon_hooks" in sys.modules:
        return
    mod = types.ModuleType("antenv.axon_hooks")
    holder = [None]
    mod.set_axon_ntff_profile_hook = lambda h: holder.__setitem__(0, h)
    mod.get_axon_ntff_profile_hook = lambda: holder[0]
    sys.modules["antenv.axon_hooks"] = mod
    try:
        if "/root/.axon_site" not in sys.path:
            sys.path.append("/root/.axon_site")
        from trn_agent_boot.trn_boot import _ntff_profile_via_ctypes

        hook = _ntff_profile_via_ctypes("/opt/axon/libaxon_pjrt.so")
        if hook is not None:
            mod.set_axon_ntff_profile_hook(hook)
    except Exception:
        pass


def _build():
    nc = bacc.Bacc("TRN2")

    xT = nc.dram_tensor("xT", [D, (NCH + PAD_BLKS) * PCOLS], BF16,
                        kind="ExternalInput")
    Wk = nc.dram_tensor("Wk", [D, U], BF16, kind="ExternalInput")
    Wu = nc.dram_tensor("Wu", [D, U], BF16, kind="ExternalInput")
    Wr = nc.dram_tensor("Wr", [U, U], BF16, kind="ExternalInput")
    Wur = nc.dram_tensor("Wur", [U, U], BF16, kind="ExternalInput")
    br = nc.dram_tensor("br", [U], F32, kind="ExternalInput")
    bur = nc.dram_tensor("bur", [U], F32, kind="ExternalInput")
    hT_out = nc.dram_tensor("hT_out", [128, MC, BL], F32, kind="ExternalOutput")
    # Swizzled step-input slabs: [chunk, m, partition(u%128), (s b)]
    xzT_d = nc.dram_tensor("xzT_d", [NCH + PAD_BLKS, MC, 128, NW], BF16)
    xhT_d = nc.dram_tensor("xhT_d", [NCH + PAD_BLKS, MC, 128, NW], BF16)

    ID = mybir.ActivationFunctionType.Identity
    SIG = mybir.ActivationFunctionType.Sigmoid
    TANH = mybir.ActivationFunctionType.Tanh

    with tile.TileContext(nc) as tc:
        with tc.tile_pool(name="consts", bufs=1) as consts:
            Wk_sb = consts.tile([128, KC, U], BF16)
            nc.sync.dma_start(Wk_sb, Wk[:, :].rearrange("(c p) u -> p c u", p=128))
            Wu_sb = consts.tile([128, KC, U], BF16)
            nc.sync.dma_start(Wu_sb, Wu[:, :].rearrange("(c p) u -> p c u", p=128))
            Wr_sb = consts.tile([128, MC, U], BF16)
            nc.sync.dma_start(Wr_sb, Wr[:, :].rearrange("(c p) u -> p c u", p=128))
            Wur_sb = consts.tile([128, MC, U], BF16)
            nc.sync.dma_start(Wur_sb, Wur[:, :].rearrange("(c p) u -> p c u", p=128))
            br_sb = consts.tile([128, MC], F32)
            nc.sync.dma_start(br_sb, br[:].rearrange("(c p) -> p c", p=128))
            bur_sb = consts.tile([128, MC], F32)
            nc.sync.dma_start(bur_sb, bur[:].rearrange("(c p) -> p c", p=128))

            hTf = consts.tile([128, MC, BL], F32)
            nc.vector.memset(hTf, 0.0)
            hTb = consts.tile([128, MC, BL], BF16)
            nc.vector.memset(hTb, 0.0)

            # Static slab tiles for the A/B chunks of each loop body.
            slabs = {}
            for nm in ("xzA", "xhA", "xzB", "xhB"):
                slabs[nm] = consts.tile([128, 1, MC, NW], BF16, name=nm)

            # ---------------- projection helpers ----------------
            def proj_block(pps, pout, xblk, c_idx, spread=None):
                """Project one chunk's xz/xh block.  When ``spread`` is a
                list, each unit of work is appended as a closure (emitted
                one per recurrence step); otherwise emitted inline."""
                def emit(fn):
                    if spread is None:
                        fn()
                    else:
                        spread.append(fn)

                for W_sb, bias_sb, dst in (
                    (Wk_sb, br_sb, xzT_d),
                    (Wu_sb, bur_sb, xhT_d),
                ):
                    for m in range(MC):
                        ps_box = []

                        def mk_mm(W_sb=W_sb, m=m, ps_box=ps_box):
                            k = len(ps_box) and ps_box[-1][1] + 1 or 0
                            if k == 0:
                                ps = pps.tile([128, PCOLS], F32, tag="projps",
                                              name="projps")
                                ps_box.append([ps, 0])
                            else:
                                ps_box[-1][1] = k
                            ps = ps_box[-1][0]
                            nc.tensor.matmul(
                                ps,
                                W_sb[:, k, m * 128 : (m + 1) * 128],
                                xblk[:, k, :],
                                start=(k == 0),
                                stop=(k == KC - 1),
                            )

                        for _ in range(KC):
                            emit(mk_mm)

                        def mk_drain(W_sb=W_sb, bias_sb=bias_sb, dst=dst,
                                     m=m, ps_box=ps_box):
                            ps = ps_box[-1][0]
                            o = pout.tile([128, PCOLS], BF16, tag="projout",
                                          name="projout")
                            nc.vector.tensor_scalar(
                                o, ps, bias_sb[:, m : m + 1], None,
                                mybir.AluOpType.add,
                            )
                            nc.sync.dma_start(
                                dst[c_idx, m, :, :], o
                            )

                        emit(mk_drain)

            # ---------------- recurrence chunk ----------------
            MH = MC // 2  # m-chunks per half

            def mm_burst(ps, W_sb_, rhs, stop_last):
                # m-outer / k-inner: accumulation groups stay consecutive
                # per PSUM slice.
                for m in range(MC):
                    for k in range(KC):
                        nc.tensor.matmul(
                            ps[:, m, :],
                            W_sb_[:, k, m * 128 : (m + 1) * 128],
                            rhs[:, k, :],
                            start=False,
                            stop=stop_last and k == KC - 1,
                        )

            def run_chunk(rps1, rps2, rtmp, xz_sb, xh_sb, spread):
                """One 64-step recurrence chunk; pops one deferred projection
                work item per step (after the step's own PE bursts)."""
                ps1 = rps1.tile([128, MC, BL], F32, tag="ps1", name="ps1h")
                nc.vector.tensor_copy(ps1, xz_sb[:, 0, :, 0:BL])
                mm_burst(ps1, Wr_sb, hTb, True)
                for s in range(S):
                    bsl = slice(s * BL, (s + 1) * BL)
                    # chain: sigmoid -> hf (bf16) -> mm2 -> tanh -> e
                    # -> next step's mm1b. The state update h' = A + e
                    # and next mm1's A-part run off the chain:
                    # z1(t+1) = xz(t+1) + A@W_r + e@W_r  (linearity).
                    fT = rtmp.tile([128, MC, BL], F32, tag="fT", name="fT")
                    nc.scalar.activation(fT, ps1, SIG)
                    hfh = rtmp.tile([128, MC, BL], BF16, tag="hf", name="hf")
                    nc.vector.tensor_mul(hfh, hTf, fT)
                    Ab = rtmp.tile([128, MC, BL], BF16, tag="Ab", name="Ab")
                    nc.vector.tensor_sub(Ab, hTf, hfh)
                    ps2 = rps2.tile([128, MC, BL], F32, tag="ps2", name="ps2")
                    nc.vector.tensor_copy(ps2, xh_sb[:, 0, :, bsl])
                    mm_burst(ps2, Wur_sb, hfh, True)
                    ps1n = None
                    if s < S - 1:
                        nsl = slice((s + 1) * BL, (s + 2) * BL)
                        ps1n = rps1.tile([128, MC, BL], F32, tag="ps1",
                                         name="ps1n")
                        nc.vector.tensor_copy(ps1n, xz_sb[:, 0, :, nsl])
                        mm_burst(ps1n, Wr_sb, Ab, False)
                    cT = rtmp.tile([128, MC, BL], F32, tag="cT", name="cT")
                    nc.scalar.activation(cT, ps2, TANH)
                    eb = rtmp.tile([128, MC, BL], BF16, tag="eb", name="eb")
                    nc.vector.tensor_mul(eb, cT, fT)
                    if s < S - 1:
                        mm_burst(ps1n, Wr_sb, eb, True)
                    nc.vector.tensor_add(hTf, Ab, eb)
                    if s == S - 1:
                        nc.vector.tensor_add(hTb, Ab, eb)
                    ps1 = ps1n
                    # deferred projection work for chunk c+2 rides in the
                    # sigmoid-wait idle window after this step's bursts.
                    # Steps 0-7 are skipped so the xT block DMA (issued at
                    # part start) is comfortably complete.
                    if spread and s >= 8:
                        spread.pop(0)()

            # ---------------- prologue: chunks 0 and 1 ----------------
            with (
                tc.tile_pool(name="proj_in", bufs=2) as pin,
                tc.tile_pool(name="proj_ps", bufs=2, space="PSUM") as pps,
                tc.tile_pool(name="proj_out", bufs=4) as pout,
                tc.tile_pool(name="rec_ps1", bufs=2, space="PSUM") as rps1,
                tc.tile_pool(name="rec_ps2", bufs=2, space="PSUM") as rps2,
                tc.tile_pool(name="rec_tmp", bufs=3) as rtmp,
            ):
                for j in range(2):
                    xblk = pin.tile([128, KC, PCOLS], BF16, tag="xblk",
                                    name="xblk")
                    nc.sync.dma_start(
                        xblk,
                        xT[:, j * PCOLS : (j + 1) * PCOLS].rearrange(
                            "(c p) n -> p c n", p=128
                        ),
                    )
                    proj_block(pps, pout, xblk, j)
                for nm, c in (("xzA", 0), ("xhA", 0), ("xzB", 1), ("xhB", 1)):
                    src = xzT_d if nm.startswith("xz") else xhT_d
                    nc.sync.dma_start(
                        slabs[nm],
                        src[c : c + 1, :, :, :].rearrange("o c p n -> p o c n"),
                    )

                # ---------------- main loop: chunk pairs ----------------
                with tc.For_i(0, NCH, 2, staggered_reset=True) as it:
                    for half, (xznm, xhnm) in enumerate(
                        (("xzA", "xhA"), ("xzB", "xhB"))
                    ):
                        # project chunk it+2+half into DRAM while recurring
                        # over chunk it+half
                        xblk = pin.tile([128, KC, PCOLS], BF16, tag="xblk",
                                        name="xblk")
                        nc.sync.dma_start(
                            xblk,
                            xT[:, bass.ds((it + 2 + half) * PCOLS, PCOLS)]
                            .rearrange("(c p) n -> p c n", p=128),
                        )
                        spread = []
                        proj_block(pps, pout, xblk, bass.ds(it + 2 + half, 1),
                                   spread=spread)
                        run_chunk(rps1, rps2, rtmp, slabs[xznm], slabs[xhnm],
                                  spread)
                        assert not spread, f"{len(spread)} leftover proj ops"
                        # reload this half's slabs for the next pair now that
                        # all its reads are done (and its projection - this
                        # body's own writes - has completed via DRAM deps).
                        for nm, src in ((xznm, xzT_d), (xhnm, xhT_d)):
                            nc.sync.dma_start(
                                slabs[nm],
                                src[bass.ds(it + 2 + half, 1), :, :, :]
                                .rearrange("o c p n -> p o c n"),
                            )

            nc.sync.dma_start(hT_out[:, :, :], hTf)

    nc.compile()
    return nc


_NC_CACHE = None


def kernel(x, W_k, W_r, b_r, W_u, W_ur, b_ur):
    global _NC_CACHE, LAST_EXEC_NS
    _install_trace_shim()
    if _NC_CACHE is None:
        _NC_CACHE = _build()
    nc = _NC_CACHE

    bf16 = ml_dtypes.bfloat16
    x = np.asarray(x, dtype=np.float32)
    Wk_b = np.ascontiguousarray(np.asarray(W_k, dtype=np.float32).astype(bf16))
    Wu_b = np.ascontiguousarray(np.asarray(W_u, dtype=np.float32).astype(bf16))
    Wr_b = np.ascontiguousarray(np.asarray(W_r, dtype=np.float32).astype(bf16))
    Wur_b = np.ascontiguousarray(np.asarray(W_ur, dtype=np.float32).astype(bf16))
    br_f = np.ascontiguousarray(np.asarray(b_r, dtype=np.float32))
    bur_f = np.ascontiguousarray(np.asarray(b_ur, dtype=np.float32))

    in_maps = []
    for c in range(NCORES):
        xc = x[c * BL : (c + 1) * BL]  # [BL, T, D]
        xTc = xc.transpose(2, 1, 0).reshape(D, T * BL).astype(bf16)
        xTp = np.zeros((D, (NCH + PAD_BLKS) * PCOLS), dtype=bf16)
        xTp[:, : T * BL] = xTc
        in_maps.append(
            {
                "xT": np.ascontiguousarray(xTp),
                "Wk": Wk_b,
                "Wu": Wu_b,
                "Wr": Wr_b,
                "Wur": Wur_b,
                "br": br_f,
                "bur": bur_f,
            }
        )

    trace = bool(os.environ.get("BASS_TRACE"))
    res = run_bass_kernel_spmd(
        nc, in_maps, core_ids=list(range(NCORES)), trace=trace
    )
    LAST_EXEC_NS = res.exec_time_ns

    out = np.empty((B, U), dtype=np.float32)
    for c in range(NCORES):
        hT = res.results[c]["hT_out"]  # [128, MC, BL]
        out[c * BL : (c + 1) * BL] = hT.transpose(2, 1, 0).reshape(BL, U)
    return out


# revision 13
# speedup vs baseline: 1.0862x; 1.0862x over previous
"""Trainium2 Bass kernel for BasicMGU (nn_BasicMGU_53386443489965).

Math (per reference):
    xz = x @ W_k ; xh = x @ W_u
    f_t = sigmoid(xz_t + h @ W_r + b_r)
    c_t = tanh(xh_t + (h*f_t) @ W_ur + b_ur)
    h   = (1-f_t)*h + f_t*c_t        -> return final h  [B, U]

Sharding: data-parallel over batch across 8 cores (B=64 -> 8 per core),
weights replicated.

Per-core design (v5):
  The input projections (two bf16 GEMMs producing bf16 xz/xh slabs) are
  FOLDED INTO the recurrence loop: while the recurrence consumes chunk c,
  the PE's idle windows project chunk c+2 (one N=512 matmul or drain per
  step, emitted after the step's bursts so it lands in the sigmoid-wait
  window).  Only chunks 0/1 are projected up front (~25us).
  The hw loop body covers a chunk PAIR (A/B), with two statically-placed
  slab tiles: A's slab for the next pair is DMA-loaded right after A's
  last read (mid-body), so chunk heads never wait on DMA, and the
  boundary-reset machinery runs half as often.
  Phase 2 (recurrence): state kept transposed hT [U(part), B(free)].
  Both per-step matmuls run weight-stationary (lhsT = 128x128 weight
  tile in bf16 -> fast weight load, rhs = state in bf16, N=B=8).
  Accumulation groups stay consecutive per PSUM slice; different PSUM
  tiles may interleave.  All compute-engine APs are static (register
  offsets cost ~100-270ns per use on the busy queues); register offsets
  appear only in DMA instructions on the sync queue.
"""

import os
import sys
import types

sys.path.insert(0, "/opt/trn_rl_repo")

import numpy as np
import ml_dtypes

import concourse.bass as bass
import concourse.mybir as mybir
import concourse.tile as tile
from concourse import bacc
from concourse.bass_utils import run_bass_kernel_spmd

B, T, D, U = 64, 1024, 512, 512
NCORES = 8
BL = B // NCORES          # batch per core
S = 64                    # recurrence steps per chunk
NCH = T // S              # chunks
KC = D // 128             # contraction chunks
MC = U // 128             # output-unit chunks
PCOLS = S * BL            # projection (t,b) columns per chunk block
NW = S * BL               # free width of one swizzled chunk slab
PAD_BLKS = 2              # scratch projection targets for the last body

F32 = mybir.dt.float32
BF16 = mybir.dt.bfloat16

LAST_EXEC_NS = None


def _install_trace_shim():
    """Make `antenv.axon_hooks` importable so trace=True degrades gracefully
    (and, where the axon .so is present, actually captures NTFF profiles)."""
    if "antenv.axon_hooks" in sys.modules:
        return
    mod = types.ModuleType("antenv.axon_hooks")
    holder = [None]
    mod.set_axon_ntff_profile_hook = lambda h: holder.__setitem__(0, h)
    mod.get_axon_ntff_profile_hook = lambda: holder[0]
    sys.modules["antenv.axon_hooks"] = mod
    try:
        if "/root/.axon_site" not in sys.path:
            sys.path.append("/root/.axon_site")
        from trn_agent_boot.trn_boot import _ntff_profile_via_ctypes

        hook = _ntff_profile_via_ctypes("/opt/axon/libaxon_pjrt.so")
        if hook is not None:
            mod.set_axon_ntff_profile_hook(hook)
    except Exception:
        pass


def _build():
    nc = bacc.Bacc("TRN2")

    xT = nc.dram_tensor("xT", [D, (NCH + PAD_BLKS) * PCOLS], BF16,
                        kind="ExternalInput")
    Wk = nc.dram_tensor("Wk", [D, U], BF16, kind="ExternalInput")
    Wu = nc.dram_tensor("Wu", [D, U], BF16, kind="ExternalInput")
    Wr = nc.dram_tensor("Wr", [U, U], BF16, kind="ExternalInput")
    Wur = nc.dram_tensor("Wur", [U, U], BF16, kind="ExternalInput")
    br = nc.dram_tensor("br", [U], F32, kind="ExternalInput")
    bur = nc.dram_tensor("bur", [U], F32, kind="ExternalInput")
    hT_out = nc.dram_tensor("hT_out", [128, MC, BL], F32, kind="ExternalOutput")

    ID = mybir.ActivationFunctionType.Identity
    SIG = mybir.ActivationFunctionType.Sigmoid
    TANH = mybir.ActivationFunctionType.Tanh

    with tile.TileContext(nc) as tc:
        with tc.tile_pool(name="consts", bufs=1) as consts:
            Wk_sb = consts.tile([128, KC, U], BF16)
            nc.sync.dma_start(Wk_sb, Wk[:, :].rearrange("(c p) u -> p c u", p=128))
            Wu_sb = consts.tile([128, KC, U], BF16)
            nc.sync.dma_start(Wu_sb, Wu[:, :].rearrange("(c p) u -> p c u", p=128))
            Wr_sb = consts.tile([128, MC, U], BF16)
            nc.sync.dma_start(Wr_sb, Wr[:, :].rearrange("(c p) u -> p c u", p=128))
            Wur_sb = consts.tile([128, MC, U], BF16)
            nc.sync.dma_start(Wur_sb, Wur[:, :].rearrange("(c p) u -> p c u", p=128))
            br_sb = consts.tile([128, MC], F32)
            nc.sync.dma_start(br_sb, br[:].rearrange("(c p) -> p c", p=128))
            bur_sb = consts.tile([128, MC], F32)
            nc.sync.dma_start(bur_sb, bur[:].rearrange("(c p) -> p c", p=128))

            hTf = consts.tile([128, MC, BL], F32)
            nc.vector.memset(hTf, 0.0)
            hTb = consts.tile([128, MC, BL], BF16)
            nc.vector.memset(hTb, 0.0)

            # Static slab tiles for the A/B chunks of each loop body, plus
            # per-half staging tiles the deferred projections drain into
            # (swapped into the slab by one SBUF->SBUF DMA once the half's
            # reads are done).
            slabs = {}
            for nm in ("xzA", "xhA", "xzB", "xhB"):
                slabs[nm] = consts.tile([128, 1, MC, NW], BF16, name=nm)
            stage = {}
            for nm in ("xzA", "xhA", "xzB", "xhB"):
                stage[nm] = consts.tile([128, MC, NW], BF16, name="st_" + nm)

            # ---------------- projection helpers ----------------
            def proj_block(pps, xblk, st_xz, st_xh, spread=None):
                """Project one chunk's xz/xh block into SBUF staging.
                When ``spread`` is a list, each unit of work is appended as
                a closure (emitted one per recurrence step); otherwise
                emitted inline."""
                def emit(fn):
                    if spread is None:
                        fn()
                    else:
                        spread.append(fn)

                for W_sb, bias_sb, dst in (
                    (Wk_sb, br_sb, st_xz),
                    (Wu_sb, bur_sb, st_xh),
                ):
                    for m in range(MC):
                        ps_box = []

                        def mk_mm(W_sb=W_sb, m=m, ps_box=ps_box):
                            k = len(ps_box) and ps_box[-1][1] + 1 or 0
                            if k == 0:
                                ps = pps.tile([128, PCOLS], F32, tag="projps",
                                              name="projps")
                                ps_box.append([ps, 0])
                            else:
                                ps_box[-1][1] = k
                            ps = ps_box[-1][0]
                            nc.tensor.matmul(
                                ps,
                                W_sb[:, k, m * 128 : (m + 1) * 128],
                                xblk[:, k, :],
                                start=(k == 0),
                                stop=(k == KC - 1),
                            )

                        for _ in range(KC):
                            emit(mk_mm)

                        def mk_drain(W_sb=W_sb, bias_sb=bias_sb, dst=dst,
                                     m=m, ps_box=ps_box):
                            ps = ps_box[-1][0]
                            if m % 2 == 0:
                                nc.scalar.activation(
                                    dst[:, m, :], ps, ID,
                                    bias=bias_sb[:, m : m + 1],
                                )
                            else:
                                nc.vector.tensor_scalar(
                                    dst[:, m, :], ps,
                                    bias_sb[:, m : m + 1], None,
                                    mybir.AluOpType.add,
                                )

                        emit(mk_drain)

            # ---------------- recurrence chunk ----------------
            MH = MC // 2  # m-chunks per half

            def mm_burst(ps, W_sb_, rhs, stop_last):
                # m-outer / k-inner: accumulation groups stay consecutive
                # per PSUM slice.
                for m in range(MC):
                    for k in range(KC):
                        nc.tensor.matmul(
                            ps[:, m, :],
                            W_sb_[:, k, m * 128 : (m + 1) * 128],
                            rhs[:, k, :],
                            start=False,
                            stop=stop_last and k == KC - 1,
                        )

            def run_chunk(rps1, rps2, rtmp, xz_sb, xh_sb, spread):
                """One 64-step recurrence chunk; pops one deferred projection
                work item per step (after the step's own PE bursts)."""
                ps1 = rps1.tile([128, MC, BL], F32, tag="ps1", name="ps1h")
                nc.vector.tensor_copy(ps1, xz_sb[:, 0, :, 0:BL])
                mm_burst(ps1, Wr_sb, hTb, True)
                for s in range(S):
                    bsl = slice(s * BL, (s + 1) * BL)
                    # chain: sigmoid -> hf (bf16) -> mm2 -> tanh -> e
                    # -> next step's mm1b. The state update h' = A + e
                    # and next mm1's A-part run off the chain:
                    # z1(t+1) = xz(t+1) + A@W_r + e@W_r  (linearity).
                    fT = rtmp.tile([128, MC, BL], F32, tag="fT", name="fT")
                    nc.scalar.activation(fT, ps1, SIG)
                    hfh = rtmp.tile([128, MC, BL], BF16, tag="hf", name="hf")
                    nc.vector.tensor_mul(hfh, hTf, fT)
                    Ab = rtmp.tile([128, MC, BL], BF16, tag="Ab", name="Ab")
                    nc.vector.tensor_sub(Ab, hTf, hfh)
                    ps2 = rps2.tile([128, MC, BL], F32, tag="ps2", name="ps2")
                    nc.vector.tensor_copy(ps2, xh_sb[:, 0, :, bsl])
                    mm_burst(ps2, Wur_sb, hfh, True)
                    ps1n = None
                    if s < S - 1:
                        nsl = slice((s + 1) * BL, (s + 2) * BL)
                        ps1n = rps1.tile([128, MC, BL], F32, tag="ps1",
                                         name="ps1n")
                        nc.vector.tensor_copy(ps1n, xz_sb[:, 0, :, nsl])
                        mm_burst(ps1n, Wr_sb, Ab, False)
                    cT = rtmp.tile([128, MC, BL], F32, tag="cT", name="cT")
                    nc.scalar.activation(cT, ps2, TANH)
                    eb = rtmp.tile([128, MC, BL], BF16, tag="eb", name="eb")
                    nc.vector.tensor_mul(eb, cT, fT)
                    if s < S - 1:
                        mm_burst(ps1n, Wr_sb, eb, True)
                    nc.vector.tensor_add(hTf, Ab, eb)
                    if s == S - 1:
                        nc.vector.tensor_add(hTb, Ab, eb)
                    ps1 = ps1n
                    # deferred projection work for chunk c+2 rides in the
                    # sigmoid-wait idle window after this step's bursts.
                    # Steps 0-7 are skipped so the xT block DMA (issued at
                    # part start) is comfortably complete.
                    if spread and s >= 8:
                        spread.pop(0)()

            # ---------------- prologue: chunks 0 and 1 ----------------
            with (
                tc.tile_pool(name="proj_in", bufs=2) as pin,
                tc.tile_pool(name="proj_ps", bufs=2, space="PSUM") as pps,
                tc.tile_pool(name="rec_ps1", bufs=2, space="PSUM") as rps1,
                tc.tile_pool(name="rec_ps2", bufs=2, space="PSUM") as rps2,
                tc.tile_pool(name="rec_tmp", bufs=3) as rtmp,
            ):
                for j, (xznm, xhnm) in enumerate(
                    (("xzA", "xhA"), ("xzB", "xhB"))
                ):
                    xblk = pin.tile([128, KC, PCOLS], BF16, tag="xblk",
                                    name="xblk")
                    nc.sync.dma_start(
                        xblk,
                        xT[:, j * PCOLS : (j + 1) * PCOLS].rearrange(
                            "(c p) n -> p c n", p=128
                        ),
                    )
                    proj_block(pps, xblk, stage[xznm], stage[xhnm])
                    for nm in (xznm, xhnm):
                        nc.sync.dma_start(slabs[nm][:, 0, :, :], stage[nm])

                # ---------------- main loop: chunk pairs ----------------
                with tc.For_i(0, NCH, 2, staggered_reset=True) as it:
                    for half, (xznm, xhnm) in enumerate(
                        (("xzA", "xhA"), ("xzB", "xhB"))
                    ):
                        # project chunk it+2+half into DRAM while recurring
                        # over chunk it+half
                        xblk = pin.tile([128, KC, PCOLS], BF16, tag="xblk",
                                        name="xblk")
                        nc.sync.dma_start(
                            xblk,
                            xT[:, bass.ds((it + 2 + half) * PCOLS, PCOLS)]
                            .rearrange("(c p) n -> p c n", p=128),
                        )
                        spread = []
                        proj_block(pps, xblk, stage[xznm], stage[xhnm],
                                   spread=spread)
                        run_chunk(rps1, rps2, rtmp, slabs[xznm], slabs[xhnm],
                                  spread)
                        assert not spread, f"{len(spread)} leftover proj ops"
                        # swap the freshly-projected chunk into this half's
                        # slab now that all its reads are done.
                        for nm in (xznm, xhnm):
                            nc.sync.dma_start(slabs[nm][:, 0, :, :], stage[nm])

            nc.sync.dma_start(hT_out[:, :, :], hTf)

    nc.compile()
    return nc


_NC_CACHE = None


def kernel(x, W_k, W_r, b_r, W_u, W_ur, b_ur):
    global _NC_CACHE, LAST_EXEC_NS
    _install_trace_shim()
    if _NC_CACHE is None:
        _NC_CACHE = _build()
    nc = _NC_CACHE

    bf16 = ml_dtypes.bfloat16
    x = np.asarray(x, dtype=np.float32)
    Wk_b = np.ascontiguousarray(np.asarray(W_k, dtype=np.float32).astype(bf16))
    Wu_b = np.ascontiguousarray(np.asarray(W_u, dtype=np.float32).astype(bf16))
    Wr_b = np.ascontiguousarray(np.asarray(W_r, dtype=np.float32).astype(bf16))
    Wur_b = np.ascontiguousarray(np.asarray(W_ur, dtype=np.float32).astype(bf16))
    br_f = np.ascontiguousarray(np.asarray(b_r, dtype=np.float32))
    bur_f = np.ascontiguousarray(np.asarray(b_ur, dtype=np.float32))

    in_maps = []
    for c in range(NCORES):
        xc = x[c * BL : (c + 1) * BL]  # [BL, T, D]
        xTc = xc.transpose(2, 1, 0).reshape(D, T * BL).astype(bf16)
        xTp = np.zeros((D, (NCH + PAD_BLKS) * PCOLS), dtype=bf16)
        xTp[:, : T * BL] = xTc
        in_maps.append(
            {
                "xT": np.ascontiguousarray(xTp),
                "Wk": Wk_b,
                "Wu": Wu_b,
                "Wr": Wr_b,
                "Wur": Wur_b,
                "br": br_f,
                "bur": bur_f,
            }
        )

    trace = bool(os.environ.get("BASS_TRACE"))
    res = run_bass_kernel_spmd(
        nc, in_maps, core_ids=list(range(NCORES)), trace=trace
    )
    LAST_EXEC_NS = res.exec_time_ns

    out = np.empty((B, U), dtype=np.float32)
    for c in range(NCORES):
        hT = res.results[c]["hT_out"]  # [128, MC, BL]
        out[c * BL : (c + 1) * BL] = hT.transpose(2, 1, 0).reshape(BL, U)
    return out
